# revision 1
# baseline (speedup 1.0000x reference)
"""HDModel retrieval kernel for 8x TRN2 NeuronCores.

reference:
    sims  = l2norm(hvs) @ l2norm(am).T        # [N, C] cosine sims
    preds = argmax(sims, axis=1)              # int32 [N]
    eta   = (sims[:,1]-sims[:,0])*0.25 + 0.5  # f32 [N]

Strategy (data-parallel over N, am replicated — no cross-core comms):
  - Host pre-transposes hvs -> hvsT [D, N/8] per shard and am -> amT [D, C]
    (layout staging only; all math happens on device).
  - sims are computed as raw = hvsT.T @ amT with f32r (tf32) matmuls,
    1 cyc/row on the PE at N>=256 vs fp32's 4.
  - am column norms (needed before argmax) via a bf16 ones-matmul over
    squared amT chunks; per-class scaling applied to sims rows on DVE.
  - hvs row norms (needed only for eta) via a bf16 gram matmul per n-tile;
    diagonal extracted with one DVE tensor_tensor_reduce against identity.
  - argmax via DVE max + max_index (top-8, index 0 = first-max like jnp).
  - preds/eta accumulate in [128, NT] tiles, one DMA out at the end;
    host reorders ([p, t] -> n = t*128+p).

This walrus build encodes ONE sync wait per TPB instruction; Tile attaches
several, so a post-pass splits multi-wait instructions into single-wait
same-engine NoOps (see _split_multiwait).
"""
import numpy as np
from contextlib import ExitStack

import concourse.bass as bass
import concourse.mybir as mybir
import concourse.tile as tile
from concourse.bass_utils import run_bass_kernel_spmd
from concourse.masks import make_identity

f32 = mybir.dt.float32
f32r = mybir.dt.float32r
bf16 = mybir.dt.bfloat16
u32 = mybir.dt.uint32

N_CORES = 8
N_FULL, D, C = 16384, 4096, 1024
NS = N_FULL // N_CORES          # 2048 rows per core
NT = NS // 128                  # 16 n-tiles
DCH = D // 128                  # 32 d-chunks
EPS = 1e-8


def _split_multiwait(nc):
    """Split multi-wait instructions into single-wait NoOps (walrus limit)."""
    ctr = [0]

    def mk_nop(engine, wait=None, update=None):
        ctr[0] += 1
        nop = mybir.InstNoOp(name=f"mwsplit_{ctr[0]}", ins=[], outs=[])
        nop.engine = engine
        nop.sync_info = mybir.SyncInfo(
            on_wait=[wait] if wait is not None else [],
            on_update=[update] if update is not None else [],
        )
        return nop

    for f in nc.m.functions:
        for bb in f.blocks:
            new = []
            changed = False
            for inst in bb.instructions:
                si = inst.sync_info
                if si is None:
                    new.append(inst)
                    continue
                waits = list(si.on_wait)
                updates = list(si.on_update)
                pre, post = [], []
                if len(waits) > 1:
                    pre = [mk_nop(inst.engine, wait=w) for w in waits[:-1]]
                    waits = waits[-1:]
                if len(updates) > 1 and type(inst).__name__ != "InstDMACopy":
                    post = [mk_nop(inst.engine, update=u) for u in updates[1:]]
                    updates = updates[:1]
                if pre or post:
                    inst.sync_info = mybir.SyncInfo(on_wait=waits, on_update=updates)
                    new.extend(pre)
                    new.append(inst)
                    new.extend(post)
                    changed = True
                else:
                    new.append(inst)
            if changed:
                bb.instructions = new


def build_nc():
    nc = bass.Bass()
    hvsT = nc.declare_dram_parameter("hvsT", [D, NS], f32r, isOutput=False)
    amT = nc.declare_dram_parameter("amT", [D, C], f32r, isOutput=False)
    ones_b = nc.declare_dram_parameter("ones_b", [128, 1], bf16, isOutput=False)
    ones_c = nc.declare_dram_parameter("ones_c", [1, 128], f32, isOutput=False)
    ident32 = nc.declare_dram_parameter("ident32", [128, 32], f32, isOutput=False)
    preds_o = nc.declare_dram_parameter("preds", [128, NT], u32, isOutput=True)
    eta_o = nc.declare_dram_parameter("eta", [128, NT], f32, isOutput=True)

    with tile.TileContext(nc) as tc, ExitStack() as ctx:
        const_p = ctx.enter_context(tc.tile_pool(name="const", bufs=1))
        am_p = ctx.enter_context(tc.tile_pool(name="am", bufs=1))
        sq_p = ctx.enter_context(tc.tile_pool(name="sq", bufs=3))
        hx_p = ctx.enter_context(tc.tile_pool(name="hx", bufs=2))
        hb_p = ctx.enter_context(tc.tile_pool(name="hb", bufs=2))
        ep_p = ctx.enter_context(tc.tile_pool(name="ep", bufs=2))
        acc_p = ctx.enter_context(tc.tile_pool(name="acc", bufs=1))
        ps_p = ctx.enter_context(tc.tile_pool(name="ps", bufs=2, space="PSUM"))
        psn_p = ctx.enter_context(tc.tile_pool(name="psn", bufs=1, space="PSUM"))

        # ---- constants ----
        ident = const_p.tile([128, 32], f32)
        nc.sync.dma_start(ident[:], ident32[:])
        ones_t = const_p.tile([128, 1], bf16)
        nc.sync.dma_start(ones_t[:], ones_b[:])
        ones_ct = const_p.tile([1, 128], f32)
        nc.sync.dma_start(ones_ct[:], ones_c[:])

        # ---- load amT (resident) ----
        am_tiles = []
        for dc in range(DCH):
            t = am_p.tile([128, C], f32r, tag=f"am{dc}")
            nc.sync.dma_start(t[:], amT[dc * 128:(dc + 1) * 128, :])
            am_tiles.append(t)

        # ---- am column norms: ones.T @ (amT**2), bf16 ----
        pn0 = psn_p.tile([1, 512], f32, tag="amn0")
        pn1 = psn_p.tile([1, 512], f32, tag="amn1")
        for dc in range(DCH):
            sq = sq_p.tile([128, C], bf16, tag="sq")
            nc.vector.tensor_mul(sq[:], am_tiles[dc][:].bitcast(f32),
                                 am_tiles[dc][:].bitcast(f32))
            nc.tensor.matmul(pn0[:], ones_t[:], sq[:, 0:512],
                             start=(dc == 0), stop=(dc == DCH - 1))
            nc.tensor.matmul(pn1[:], ones_t[:], sq[:, 512:C],
                             start=(dc == 0), stop=(dc == DCH - 1))

        # inv_c = 1 / max(sqrt(normsq), EPS), laid out [1, C] on partition 0
        amn = const_p.tile([1, C], f32)
        nc.scalar.sqrt(amn[:, 0:512], pn0[:])
        nc.scalar.sqrt(amn[:, 512:C], pn1[:])
        nc.vector.tensor_scalar_max(amn[:], amn[:], EPS)
        inv_c = const_p.tile([1, C], f32)
        nc.vector.reciprocal(inv_c[:], amn[:])

        # broadcast inv_c to all 128 partitions via exact fp32 ones-outer-product
        inv_cb = const_p.tile([128, C], f32)
        for h in range(2):
            bc = ps_p.tile([128, 512], f32, tag=("psA" if h == 0 else "psB"))
            nc.tensor.matmul(bc[:], ones_ct[:], inv_c[:, h * 512:(h + 1) * 512],
                             start=True, stop=True)
            nc.scalar.copy(inv_cb[:, h * 512:(h + 1) * 512], bc[:])

        # ---- accumulators ----
        preds_acc = acc_p.tile([128, NT], u32)
        eta_acc = acc_p.tile([128, NT], f32)

        # ---- main loop over n-tiles ----
        for t in range(NT):
            hx = hx_p.tile([128, D], f32r, tag="hx")
            src = hvsT[:, t * 128:(t + 1) * 128].rearrange(
                "(dc p) j -> p dc j", p=128)
            hxv = hx[:].rearrange("p (dc j) -> p dc j", j=128)
            half = DCH // 2
            nc.sync.dma_start(hxv[:, 0:half, :], src[:, 0:half, :])
            nc.sync.dma_start(hxv[:, half:DCH, :], src[:, half:DCH, :])

            hb = hb_p.tile([128, D], bf16, tag="hb")
            nc.scalar.copy(hb[:, 0:D // 2], hx[:, 0:D // 2].bitcast(f32))
            nc.scalar.copy(hb[:, D // 2:D], hx[:, D // 2:D].bitcast(f32))

            pA = ps_p.tile([128, 512], f32, tag="psA")
            pB = ps_p.tile([128, 512], f32, tag="psB")
            pG = ps_p.tile([128, 32], f32, tag="psG")
            for dc in range(DCH):
                lhs = hx[:, dc * 128:(dc + 1) * 128]
                nc.tensor.matmul(pA[:], lhs, am_tiles[dc][:, 0:512],
                                 start=(dc == 0), stop=(dc == DCH - 1))
                nc.tensor.matmul(pB[:], lhs, am_tiles[dc][:, 512:C],
                                 start=(dc == 0), stop=(dc == DCH - 1))
            # 4 col-packed 32-wide gram blocks run concurrently in the PE
            # array (tile_position col-tiling); only the diagonal is needed.
            for dc in range(DCH):
                for b in range(4):
                    sl = hb[:, dc * 128 + 32 * b:dc * 128 + 32 * (b + 1)]
                    nc.tensor.matmul(pG[32 * b:32 * (b + 1), :], sl, sl,
                                     start=(dc == 0), stop=(dc == DCH - 1),
                                     tile_position=(0, 32 * b))

            # epilogue
            sc = ep_p.tile([128, C], f32, tag="sc")
            nc.vector.tensor_mul(sc[:, 0:512], pA[:], inv_cb[:, 0:512])
            nc.vector.tensor_mul(sc[:, 512:C], pB[:], inv_cb[:, 512:C])

            dg = ep_p.tile([128, 32], f32, tag="dg")
            nsq = ep_p.tile([128, 1], f32, tag="nsq")
            nc.vector.tensor_mul(dg[:], pG[:], ident[:])
            nc.vector.reduce_sum(nsq[:], dg[:], axis=mybir.AxisListType.X)
            nrm = ep_p.tile([128, 1], f32, tag="nrm")
            nc.scalar.sqrt(nrm[:], nsq[:])
            nc.vector.tensor_scalar_max(nrm[:], nrm[:], EPS)
            inv_n = ep_p.tile([128, 1], f32, tag="invn")
            nc.vector.reciprocal(inv_n[:], nrm[:])

            mx = ep_p.tile([128, 8], f32, tag="mx")
            ix = ep_p.tile([128, 8], u32, tag="ix")
            nc.vector.max(out=mx[:], in_=sc[:])
            nc.vector.max_index(out=ix[:], in_max=mx[:], in_values=sc[:])
            nc.vector.tensor_copy(preds_acc[:, t:t + 1], ix[:, 0:1])

            d01 = ep_p.tile([128, 1], f32, tag="d01")
            nc.vector.tensor_sub(d01[:], sc[:, 1:2], sc[:, 0:1])
            nc.vector.tensor_mul(d01[:], d01[:], inv_n[:])
            nc.vector.tensor_scalar(
                out=eta_acc[:, t:t + 1], in0=d01[:], scalar1=0.25, scalar2=0.5,
                op0=mybir.AluOpType.mult, op1=mybir.AluOpType.add)

        nc.sync.dma_start(preds_o[:], preds_acc[:])
        nc.sync.dma_start(eta_o[:], eta_acc[:])

    _split_multiwait(nc)
    return nc


_CACHE = {}


def kernel(hvs: np.ndarray, am: np.ndarray):
    hvs = np.asarray(hvs, dtype=np.float32)
    am = np.asarray(am, dtype=np.float32)
    assert hvs.shape == (N_FULL, D) and am.shape == (C, D)

    if "nc" not in _CACHE:
        _CACHE["nc"] = build_nc()
    nc = _CACHE["nc"]

    amT = np.ascontiguousarray(am.T)                      # [D, C]
    import ml_dtypes
    ones_b = np.ones((128, 1), dtype=ml_dtypes.bfloat16)
    ones_c = np.ones((1, 128), dtype=np.float32)
    ident32 = np.zeros((128, 32), dtype=np.float32)
    for b in range(4):
        ident32[32 * b:32 * (b + 1), :] = np.eye(32, dtype=np.float32)

    in_maps = []
    for r in range(N_CORES):
        shard = hvs[r * NS:(r + 1) * NS]                  # [NS, D]
        hvsT = np.ascontiguousarray(shard.T)              # [D, NS]
        in_maps.append({"hvsT": hvsT, "amT": amT, "ones_b": ones_b,
                        "ones_c": ones_c, "ident32": ident32})

    res = run_bass_kernel_spmd(nc, in_maps, core_ids=list(range(N_CORES)))

    preds = np.empty(N_FULL, dtype=np.int32)
    eta = np.empty(N_FULL, dtype=np.float32)
    for r in range(N_CORES):
        p = res.results[r]["preds"]                       # [128, NT] u32
        e = res.results[r]["eta"]                         # [128, NT] f32
        preds[r * NS:(r + 1) * NS] = p.T.ravel().astype(np.int32)
        eta[r * NS:(r + 1) * NS] = e.T.ravel()
    return preds, eta



# revision 2
# speedup vs baseline: 1.1727x; 1.1727x over previous
"""HDModel retrieval kernel for 8x TRN2 NeuronCores — v2.

reference:
    sims  = l2norm(hvs) @ l2norm(am).T        # [N, C] cosine sims
    preds = argmax(sims, axis=1)              # int32 [N]
    eta   = (sims[:,1]-sims[:,0])*0.25 + 0.5  # f32 [N]

Data-parallel over N, am replicated, no cross-core comms.

v2 changes vs baseline (309 us):
  - PE runs ONLY the f32r sims matmuls (the ~219 us roofline). Norm work
    moves off the PE:
      am col norms:  Act square (bf16) -> DVE f32 accumulate -> Pool
                     partition_all_reduce [128,1024] (result lands
                     replicated across partitions = the broadcast layout
                     the epilogue needs) -> sqrt / max-eps / reciprocal.
      row norms:     Act square (bf16) -> DVE grouped adds down to
                     [128 d, 128 n] -> Pool partition_all_reduce
                     [128,128] -> 4 diagonal 32x32 StreamTransposes to
                     land nsq on the n-partition axis -> sqrt/max/recip.
  - Startup: the serialized am DMA (~50 us) overlaps a staggered 3-tile
    chunk rotation; late tiles wrap around to their missed chunks (psum
    accumulation order is irrelevant).
  - Epilogue scales psum in place (no [128,1024] sims sbuf tile), does
    per-half max/max_index and a compare/select merge (tie prefers half
    A = lower class index, matching jnp.argmax first-max semantics).

This walrus build encodes ONE sync wait per TPB instruction; Tile attaches
several, so a post-pass splits multi-wait instructions into single-wait
same-engine NoOps (see _split_multiwait).
"""
import numpy as np
from contextlib import ExitStack

import concourse.bass as bass
import concourse.mybir as mybir
import concourse.tile as tile
from concourse.bass_utils import run_bass_kernel_spmd

f32 = mybir.dt.float32
f32r = mybir.dt.float32r
bf16 = mybir.dt.bfloat16
u32 = mybir.dt.uint32

N_CORES = 8
N_FULL, D, C = 16384, 4096, 1024
NS = N_FULL // N_CORES          # 2048 rows per core
NT = NS // 128                  # 16 n-tiles
DCH = D // 128                  # 32 d-chunks
EPS = 1e-8
ROT = 3                         # tiles in the startup rotation
ENTRY = [0, 2, 6]               # rotation entry chunk per tile
HX_AFTER = {1: 1, 2: 5}         # dma hx[t] right after am chunk HX_AFTER[t]
PRE_AT = {2: 0, 7: 1, 11: 2}    # am chunk -> tile: emit rownorm_pre here


def _split_multiwait(nc):
    """Split multi-wait instructions into single-wait NoOps (walrus limit)."""
    ctr = [0]

    def mk_nop(engine, wait=None, update=None):
        ctr[0] += 1
        nop = mybir.InstNoOp(name=f"mwsplit_{ctr[0]}", ins=[], outs=[])
        nop.engine = engine
        nop.sync_info = mybir.SyncInfo(
            on_wait=[wait] if wait is not None else [],
            on_update=[update] if update is not None else [],
        )
        return nop

    for f in nc.m.functions:
        for bb in f.blocks:
            new = []
            changed = False
            for inst in bb.instructions:
                si = inst.sync_info
                if si is None:
                    new.append(inst)
                    continue
                waits = list(si.on_wait)
                updates = list(si.on_update)
                pre, post = [], []
                if len(waits) > 1:
                    pre = [mk_nop(inst.engine, wait=w) for w in waits[:-1]]
                    waits = waits[-1:]
                if len(updates) > 1 and type(inst).__name__ != "InstDMACopy":
                    post = [mk_nop(inst.engine, update=u) for u in updates[1:]]
                    updates = updates[:1]
                if pre or post:
                    inst.sync_info = mybir.SyncInfo(on_wait=waits, on_update=updates)
                    new.extend(pre)
                    new.append(inst)
                    new.extend(post)
                    changed = True
                else:
                    new.append(inst)
            if changed:
                bb.instructions = new


def build_nc():
    nc = bass.Bass()
    hvsT = nc.declare_dram_parameter("hvsT", [D, NS], f32r, isOutput=False)
    amT = nc.declare_dram_parameter("amT", [D, C], f32r, isOutput=False)
    preds_o = nc.declare_dram_parameter("preds", [128, NT], u32, isOutput=True)
    eta_o = nc.declare_dram_parameter("eta", [128, NT], f32, isOutput=True)

    with tile.TileContext(nc) as tc, ExitStack() as ctx:
        am_p = ctx.enter_context(tc.tile_pool(name="am", bufs=1))
        hx_p = ctx.enter_context(tc.tile_pool(name="hx", bufs=3))
        sq_p = ctx.enter_context(tc.tile_pool(name="sq", bufs=1))
        sqam_p = ctx.enter_context(tc.tile_pool(name="sqam", bufs=2))
        nrm_p = ctx.enter_context(tc.tile_pool(name="nrm", bufs=1))
        rn_p = ctx.enter_context(tc.tile_pool(name="rn", bufs=2))
        rna_p = ctx.enter_context(tc.tile_pool(name="rna", bufs=4))
        acc_p = ctx.enter_context(tc.tile_pool(name="acc", bufs=1))
        psA_p = ctx.enter_context(tc.tile_pool(name="psA", bufs=3, space="PSUM"))
        psB_p = ctx.enter_context(tc.tile_pool(name="psB", bufs=3, space="PSUM"))
        pn_p = ctx.enter_context(tc.tile_pool(name="pn", bufs=1, space="PSUM"))

        inv_cb = nrm_p.tile([128, C], f32)        # 1/|am_c|, all partitions
        inv_c1 = nrm_p.tile([1, C], f32)          # staging row for broadcast
        ones_b = nrm_p.tile([128, 1], bf16)       # matmul reduction vector
        ones_c = nrm_p.tile([1, 128], f32)        # broadcast outer-product lhs
        preds_acc = acc_p.tile([128, NT], u32)
        eta_acc = acc_p.tile([128, NT], f32)
        nc.vector.memset(ones_b[:], 1.0)
        nc.vector.memset(ones_c[:], 1.0)

        am_tiles = []
        hx_tiles = {}

        def load_hx(t):
            """DMA one n-tile of hvsT into [d-part, (dc, n)] layout, quartered."""
            hx = hx_p.tile([128, D], f32r, tag="hx", name=f"hx{t}")
            src = hvsT[:, t * 128:(t + 1) * 128].rearrange(
                "(dc p) j -> p dc j", p=128)
            hxv = hx[:].rearrange("p (dc j) -> p dc j", j=128)
            q = DCH // 4
            for k in range(4):
                nc.sync.dma_start(hxv[:, k * q:(k + 1) * q, :],
                                  src[:, k * q:(k + 1) * q, :])
            hx_tiles[t] = hx

        def load_am(dc):
            t = am_p.tile([128, C], f32r, name=f"am{dc}")
            nc.sync.dma_start(t[:], amT[dc * 128:(dc + 1) * 128, :])
            am_tiles.append(t)

        # full-height tiles: row 0 holds the column-sum accumulation; after
        # the sqrt/recip read, the same banks take the broadcast outer product
        pn0 = pn_p.tile([128, 512], f32, tag="pn0", name="pn0")
        pn1 = pn_p.tile([128, 512], f32, tag="pn1", name="pn1")
        sq_pairs = {}

        def amnorm_chunk(dc):
            """Act square; on odd chunks pair-add (bf16) and accumulate the
            pair into pn via two 512-wide bf16 ones-matmuls (0.43 us PE per
            pair ~= the rotation's per-chunk slack vs the am arrival rate)."""
            sq = sqam_p.tile([128, C], bf16, tag="sqam", name=f"sqam{dc}")
            nc.scalar.activation(out=sq[:], in_=am_tiles[dc][:].bitcast(f32),
                                 func=mybir.ActivationFunctionType.Square)
            sq_pairs[dc] = sq
            if dc % 2 == 1:
                pair = sq_pairs.pop(dc - 1)
                nc.vector.tensor_add(pair[:], pair[:], sq[:])
                k = dc // 2
                if k == DCH // 2 - 1:
                    sq_pairs['last'] = pair   # emit after the rounds: keeps
                    return                    # round 31 off this chunk's
                nc.tensor.matmul(pn0[0:1, :], ones_b[:], pair[:, 0:512],     # Act/DVE chain
                                 start=(k == 0), stop=False)
                nc.tensor.matmul(pn1[0:1, :], ones_b[:], pair[:, 512:C],
                                 start=(k == 0), stop=False)

        def rownorm_pre(t):
            """Act square + DVE grouped adds -> sqacc [128 d, 128 n] bf16."""
            hx = hx_tiles[t]
            sq = sq_p.tile([128, D // 2], bf16, tag="sqhx", name=f"sqhx{t}")
            sa = rn_p.tile([128, 512], bf16, tag="rnsa", name=f"rnsa{t}")
            sqacc = rna_p.tile([128, 128], bf16, tag="rnacc", name=f"rnacc{t}")
            for h in range(2):
                half = hx[:, h * (D // 2):(h + 1) * (D // 2)].bitcast(f32)
                nc.scalar.activation(out=sq[:], in_=half,
                                     func=mybir.ActivationFunctionType.Square)
                if h == 0:
                    nc.vector.tensor_add(sa[:], sq[:, 0:512], sq[:, 512:1024])
                else:
                    nc.vector.tensor_add(sa[:], sa[:], sq[:, 0:512])
                    nc.vector.tensor_add(sa[:], sa[:], sq[:, 512:1024])
                nc.vector.tensor_add(sa[:], sa[:], sq[:, 1024:1536])
                nc.vector.tensor_add(sa[:], sa[:], sq[:, 1536:2048])
            nc.vector.tensor_add(sqacc[:], sa[:, 0:128], sa[:, 128:256])
            nc.vector.tensor_add(sqacc[:], sqacc[:], sa[:, 256:384])
            nc.vector.tensor_add(sqacc[:], sqacc[:], sa[:, 384:512])
            return sqacc

        def rownorm_post(t, sqacc):
            """Full 128x128 block transpose (d<->n) then free-axis reduce:
            a partition reduction built only from DVE ops."""
            red = rn_p.tile([128, 128], bf16, tag="rnred", name=f"rnred{t}")
            for i in range(4):
                for j in range(4):
                    nc.vector.transpose(
                        red[i * 32:(i + 1) * 32, j * 32:(j + 1) * 32],
                        sqacc[j * 32:(j + 1) * 32, i * 32:(i + 1) * 32])
            nsq = rn_p.tile([128, 1], f32, tag="rnt", name=f"rnt{t}")
            nc.vector.reduce_sum(nsq[:], red[:], axis=mybir.AxisListType.X)
            inv_n = rn_p.tile([128, 1], f32, tag="invn", name=f"invn{t}")
            nc.scalar.sqrt(inv_n[:], nsq[:])
            nc.vector.reciprocal(inv_n[:], inv_n[:])
            return inv_n

        def epilogue(t, pA, pB, inv_n):
            ep = rn_p.tile([128, 24], f32, tag="ep", name=f"ep{t}")
            mxA, mxB = ep[:, 0:8], ep[:, 8:16]
            d01, c01 = ep[:, 16:17], ep[:, 18:20]
            epi = rn_p.tile([128, 24], u32, tag="epi", name=f"epi{t}")
            ixA, ixB, msk = epi[:, 0:8], epi[:, 8:16], epi[:, 16:17]
            # half A first, releasing the pA psum slot as early as possible
            # (the next-but-two tile's matmuls recycle it); DVE may read only
            # one PSUM operand per op, so cols 0:2 stage through SBUF
            nc.vector.tensor_mul(pA[:], pA[:], inv_cb[:, 0:512])
            nc.vector.tensor_copy(c01, pA[:, 0:2])
            nc.vector.max(out=mxA, in_=pA[:])
            nc.vector.max_index(out=ixA, in_max=mxA, in_values=pA[:])
            nc.vector.tensor_mul(pB[:], pB[:], inv_cb[:, 512:C])
            nc.vector.max(out=mxB, in_=pB[:])
            nc.vector.max_index(out=ixB, in_max=mxB, in_values=pB[:])
            nc.vector.tensor_sub(d01, c01[:, 1:2], c01[:, 0:1])
            nc.vector.tensor_mul(d01, d01, inv_n[:])
            nc.vector.tensor_scalar(
                out=eta_acc[:, t:t + 1], in0=d01, scalar1=0.25, scalar2=0.5,
                op0=mybir.AluOpType.mult, op1=mybir.AluOpType.add)
            nc.vector.tensor_scalar_add(ixB[:, 0:1], ixB[:, 0:1], 512)
            nc.vector.tensor_tensor(out=msk, in0=mxA[:, 0:1], in1=mxB[:, 0:1],
                                    op=mybir.AluOpType.is_ge)
            nc.vector.tensor_copy(preds_acc[:, t:t + 1], ixB[:, 0:1])
            nc.vector.copy_predicated(preds_acc[:, t:t + 1], msk, ixA[:, 0:1])

        def alloc_ps(t):
            if t == ROT:
                # the pn banks are dead once inv_cb is broadcast (just before
                # the rotation ends), so the first phase-2 tile accumulates
                # there instead of waiting for a rotation tile's epilogue
                pA = pn_p.tile([128, 512], f32, tag="pn0", name=f"pA{t}")
                pB = pn_p.tile([128, 512], f32, tag="pn1", name=f"pB{t}")
            else:
                pA = psA_p.tile([128, 512], f32, tag="pA", name=f"pA{t}")
                pB = psB_p.tile([128, 512], f32, tag="pB", name=f"pB{t}")
            return pA, pB

        def mm_pair(t, dc, first, last):
            pA, pB = rot_ps[t]
            lhs = hx_tiles[t][:, dc * 128:(dc + 1) * 128]
            nc.tensor.matmul(pA[:], lhs, am_tiles[dc][:, 0:512],
                             start=first, stop=last)
            nc.tensor.matmul(pB[:], lhs, am_tiles[dc][:, 512:C],
                             start=first, stop=last)

        # ================= phase 1: staggered rotation =================
        # DMA order (one serial resource): am0, hx0, am1, hx1, am2..5, hx2,
        # am6..31. Act/DVE norm work and PE matmul rounds are emitted in the
        # same chunk order so every engine's in-order stream is paced by the
        # arrival of its own inputs.
        load_am(0)
        load_hx(0)
        rot_ps = {}
        rot_sqacc = {}
        for t in range(ROT):
            rot_ps[t] = alloc_ps(t)
        for dc in range(DCH):
            if dc > 0:
                load_am(dc)
            for t, after in HX_AFTER.items():
                if after == dc:
                    load_hx(t)
            amnorm_chunk(dc)
            if dc in PRE_AT:
                t_pre = PRE_AT[dc]
                rot_sqacc[t_pre] = rownorm_pre(t_pre)
            for t in range(ROT):
                if dc == ENTRY[t]:
                    mm_pair(t, dc, first=True, last=False)
                    for wdc in range(ENTRY[t]):   # wrap: missed chunks are
                        mm_pair(t, wdc, first=False, last=False)  # resident
                elif dc > ENTRY[t]:
                    mm_pair(t, dc, first=False, last=(dc == DCH - 1))

        pair = sq_pairs.pop('last')
        nc.tensor.matmul(pn0[0:1, :], ones_b[:], pair[:, 0:512],
                         start=False, stop=True)
        nc.tensor.matmul(pn1[0:1, :], ones_b[:], pair[:, 512:C],
                         start=False, stop=True)
        # ---- finish am norms -> inv_cb (halves pipelined) ----
        for h, pn in ((0, pn0), (1, pn1)):
            cols = slice(h * 512, (h + 1) * 512)
            nc.scalar.sqrt(inv_c1[:, cols], pn[0:1, :])
            nc.vector.reciprocal(inv_c1[:, cols], inv_c1[:, cols])
            nc.tensor.matmul(pn[:], ones_c[:], inv_c1[:, cols],
                             start=True, stop=True)
            nc.vector.tensor_copy(inv_cb[:, cols], pn[:])

        # rotation tiles: rownorm back half + epilogue
        for t in range(ROT):
            inv_n = rownorm_post(t, rot_sqacc[t])
            pA, pB = rot_ps[t]
            epilogue(t, pA, pB, inv_n)

        # ================= phase 2: serial tiles =================
        for t in range(ROT, NT):
            load_hx(t)
            sqacc = rownorm_pre(t)
            inv_n = rownorm_post(t, sqacc)
            rot_ps[t] = alloc_ps(t)
            for dc in range(DCH):
                mm_pair(t, dc, first=(dc == 0), last=(dc == DCH - 1))
            epilogue(t, *rot_ps[t], inv_n)
            if t == 7:
                nc.sync.dma_start(preds_o[:, 0:8], preds_acc[:, 0:8])
                nc.sync.dma_start(eta_o[:, 0:8], eta_acc[:, 0:8])

        nc.sync.dma_start(preds_o[:, 8:NT], preds_acc[:, 8:NT])
        nc.sync.dma_start(eta_o[:, 8:NT], eta_acc[:, 8:NT])

    _split_multiwait(nc)
    return nc


_CACHE = {}


def kernel(hvs: np.ndarray, am: np.ndarray):
    hvs = np.asarray(hvs, dtype=np.float32)
    am = np.asarray(am, dtype=np.float32)
    assert hvs.shape == (N_FULL, D) and am.shape == (C, D)

    if "nc" not in _CACHE:
        _CACHE["nc"] = build_nc()
    nc = _CACHE["nc"]

    amT = np.ascontiguousarray(am.T)                      # [D, C]
    in_maps = []
    for r in range(N_CORES):
        shard = hvs[r * NS:(r + 1) * NS]                  # [NS, D]
        hvsT = np.ascontiguousarray(shard.T)              # [D, NS]
        in_maps.append({"hvsT": hvsT, "amT": amT})

    res = run_bass_kernel_spmd(nc, in_maps, core_ids=list(range(N_CORES)))

    preds = np.empty(N_FULL, dtype=np.int32)
    eta = np.empty(N_FULL, dtype=np.float32)
    for r in range(N_CORES):
        p = res.results[r]["preds"]                       # [128, NT] u32
        e = res.results[r]["eta"]                         # [128, NT] f32
        preds[r * NS:(r + 1) * NS] = p.T.ravel().astype(np.int32)
        eta[r * NS:(r + 1) * NS] = e.T.ravel()
    return preds, eta


# revision 3
# speedup vs baseline: 1.1754x; 1.0023x over previous
"""HDModel retrieval kernel for 8x TRN2 NeuronCores.

reference:
    sims  = l2norm(hvs) @ l2norm(am).T        # [N, C] cosine sims
    preds = argmax(sims, axis=1)              # int32 [N]
    eta   = (sims[:,1]-sims[:,0])*0.25 + 0.5  # f32 [N]

Data-parallel over N, am replicated, no cross-core comms.
309 us (previous baseline) -> ~263 us.

Design (engine budget: PE sims floor is 16 tiles x 64 f32r matmuls x
512 cols x 0.42 ns = ~219 us; everything else hides behind it):
  - PE runs the f32r sims matmuls plus only ~7 us of am-norm column
    reduction (bf16 ones-matmuls over pair-summed squares) and two fp32
    outer-product broadcasts. The old per-tile gram matmuls (~27 us) and
    full per-chunk norm matmuls (~14 us) are gone.
  - am col norms: Act squares each arriving am chunk (bf16); DVE adds
    chunk pairs; a [1,512]-row ones-matmul accumulates the pairs into two
    psum banks; after the last chunk: Act sqrt -> DVE reciprocal -> fp32
    ones outer-product rebroadcasts 1/|am_c| to all 128 partitions.
  - row norms: Act squares hx halves (bf16) -> DVE grouped adds down to
    sqacc [128 d, 128 n] -> full 128x128 transpose as 16 StreamTranspose
    32x32 blocks (a partition reduction built from DVE ops only; the
    gpsimd partition_all_reduce ucode does not compile on this backend)
    -> free-axis reduce -> sqrt -> reciprocal = 1/|x_n| on n-partitions.
  - Startup: the serialized DMA stream (am is 50 us of it) overlaps a
    staggered 3-tile chunk rotation (entries 0/2/6); a tile entering at
    round E first back-fills its missed chunks 0..E-1, which are already
    resident (psum accumulation order is irrelevant). The first phase-2
    tile accumulates in the pn banks, dead after the inv_c broadcast, so
    it starts without waiting for any rotation epilogue (this also keeps
    the PE p-state ramp warm into phase 2).
  - Epilogue scales psum in place (no [128,1024] sims sbuf tile), does
    per-half max/max_index and a compare/select merge (tie prefers half
    A = lower class index, matching jnp.argmax first-max semantics).
    eta's columns 0,1 stage through SBUF (single-PSUM-read rule).

This walrus build encodes ONE sync wait per TPB instruction; Tile attaches
several, so a post-pass splits multi-wait instructions into single-wait
same-engine NoOps (see _split_multiwait).
"""
import numpy as np
from contextlib import ExitStack

import concourse.bass as bass
import concourse.mybir as mybir
import concourse.tile as tile
from concourse.bass_utils import run_bass_kernel_spmd

f32 = mybir.dt.float32
f32r = mybir.dt.float32r
bf16 = mybir.dt.bfloat16
u32 = mybir.dt.uint32

N_CORES = 8
N_FULL, D, C = 16384, 4096, 1024
NS = N_FULL // N_CORES          # 2048 rows per core
NT = NS // 128                  # 16 n-tiles
DCH = D // 128                  # 32 d-chunks
EPS = 1e-8
ROT = 3                         # tiles in the startup rotation
ENTRY = [0, 2, 6]               # rotation entry chunk per tile
HX_AFTER = {1: 1, 2: 5}         # dma hx[t] right after am chunk HX_AFTER[t]
PRE_AT = {2: 0, 7: 1, 11: 2}    # am chunk -> tile: emit rownorm_pre here


def _split_multiwait(nc):
    """Split multi-wait instructions into single-wait NoOps (walrus limit)."""
    ctr = [0]

    def mk_nop(engine, wait=None, update=None):
        ctr[0] += 1
        nop = mybir.InstNoOp(name=f"mwsplit_{ctr[0]}", ins=[], outs=[])
        nop.engine = engine
        nop.sync_info = mybir.SyncInfo(
            on_wait=[wait] if wait is not None else [],
            on_update=[update] if update is not None else [],
        )
        return nop

    for f in nc.m.functions:
        for bb in f.blocks:
            new = []
            changed = False
            for inst in bb.instructions:
                si = inst.sync_info
                if si is None:
                    new.append(inst)
                    continue
                waits = list(si.on_wait)
                updates = list(si.on_update)
                pre, post = [], []
                if len(waits) > 1:
                    pre = [mk_nop(inst.engine, wait=w) for w in waits[:-1]]
                    waits = waits[-1:]
                if len(updates) > 1 and type(inst).__name__ != "InstDMACopy":
                    post = [mk_nop(inst.engine, update=u) for u in updates[1:]]
                    updates = updates[:1]
                if pre or post:
                    inst.sync_info = mybir.SyncInfo(on_wait=waits, on_update=updates)
                    new.extend(pre)
                    new.append(inst)
                    new.extend(post)
                    changed = True
                else:
                    new.append(inst)
            if changed:
                bb.instructions = new


def build_nc():
    nc = bass.Bass()
    hvsT = nc.declare_dram_parameter("hvsT", [D, NS], f32r, isOutput=False)
    amT = nc.declare_dram_parameter("amT", [D, C], f32r, isOutput=False)
    preds_o = nc.declare_dram_parameter("preds", [128, NT], u32, isOutput=True)
    eta_o = nc.declare_dram_parameter("eta", [128, NT], f32, isOutput=True)

    with tile.TileContext(nc) as tc, ExitStack() as ctx:
        am_p = ctx.enter_context(tc.tile_pool(name="am", bufs=1))
        hx_p = ctx.enter_context(tc.tile_pool(name="hx", bufs=3))
        sq_p = ctx.enter_context(tc.tile_pool(name="sq", bufs=1))
        sqam_p = ctx.enter_context(tc.tile_pool(name="sqam", bufs=2))
        nrm_p = ctx.enter_context(tc.tile_pool(name="nrm", bufs=1))
        rn_p = ctx.enter_context(tc.tile_pool(name="rn", bufs=2))
        rna_p = ctx.enter_context(tc.tile_pool(name="rna", bufs=4))
        acc_p = ctx.enter_context(tc.tile_pool(name="acc", bufs=1))
        psA_p = ctx.enter_context(tc.tile_pool(name="psA", bufs=3, space="PSUM"))
        psB_p = ctx.enter_context(tc.tile_pool(name="psB", bufs=3, space="PSUM"))
        pn_p = ctx.enter_context(tc.tile_pool(name="pn", bufs=1, space="PSUM"))

        inv_cb = nrm_p.tile([128, C], f32)        # 1/|am_c|, all partitions
        inv_c1 = nrm_p.tile([1, C], f32)          # staging row for broadcast
        ones_b = nrm_p.tile([128, 1], bf16)       # matmul reduction vector
        ones_c = nrm_p.tile([1, 128], f32)        # broadcast outer-product lhs
        preds_acc = acc_p.tile([128, NT], u32)
        eta_acc = acc_p.tile([128, NT], f32)
        nc.vector.memset(ones_b[:], 1.0)
        nc.vector.memset(ones_c[:], 1.0)

        am_tiles = []
        hx_tiles = {}

        def load_hx(t):
            """DMA one n-tile of hvsT into [d-part, (dc, n)] layout, quartered."""
            hx = hx_p.tile([128, D], f32r, tag="hx", name=f"hx{t}")
            src = hvsT[:, t * 128:(t + 1) * 128].rearrange(
                "(dc p) j -> p dc j", p=128)
            hxv = hx[:].rearrange("p (dc j) -> p dc j", j=128)
            q = DCH // 4
            for k in range(4):
                nc.sync.dma_start(hxv[:, k * q:(k + 1) * q, :],
                                  src[:, k * q:(k + 1) * q, :])
            hx_tiles[t] = hx

        def load_am(dc):
            t = am_p.tile([128, C], f32r, name=f"am{dc}")
            nc.sync.dma_start(t[:], amT[dc * 128:(dc + 1) * 128, :])
            am_tiles.append(t)

        # full-height tiles: row 0 holds the column-sum accumulation; after
        # the sqrt/recip read, the same banks take the broadcast outer product
        pn0 = pn_p.tile([128, 512], f32, tag="pn0", name="pn0")
        pn1 = pn_p.tile([128, 512], f32, tag="pn1", name="pn1")
        sq_pairs = {}

        def amnorm_chunk(dc):
            """Act square; on odd chunks pair-add (bf16) and accumulate the
            pair into pn via two 512-wide bf16 ones-matmuls (0.43 us PE per
            pair ~= the rotation's per-chunk slack vs the am arrival rate)."""
            sq = sqam_p.tile([128, C], bf16, tag="sqam", name=f"sqam{dc}")
            nc.scalar.activation(out=sq[:], in_=am_tiles[dc][:].bitcast(f32),
                                 func=mybir.ActivationFunctionType.Square)
            sq_pairs[dc] = sq
            if dc % 2 == 1:
                pair = sq_pairs.pop(dc - 1)
                nc.vector.tensor_add(pair[:], pair[:], sq[:])
                k = dc // 2
                if k == DCH // 2 - 1:
                    sq_pairs['last'] = pair   # emit after the rounds: keeps
                    return                    # round 31 off this chunk's
                nc.tensor.matmul(pn0[0:1, :], ones_b[:], pair[:, 0:512],     # Act/DVE chain
                                 start=(k == 0), stop=False)
                nc.tensor.matmul(pn1[0:1, :], ones_b[:], pair[:, 512:C],
                                 start=(k == 0), stop=False)

        def rownorm_pre(t):
            """Act square + DVE grouped adds -> sqacc [128 d, 128 n] bf16."""
            hx = hx_tiles[t]
            sq = sq_p.tile([128, D // 2], bf16, tag="sqhx", name=f"sqhx{t}")
            sa = rn_p.tile([128, 512], bf16, tag="rnsa", name=f"rnsa{t}")
            sqacc = rna_p.tile([128, 128], bf16, tag="rnacc", name=f"rnacc{t}")
            for h in range(2):
                half = hx[:, h * (D // 2):(h + 1) * (D // 2)].bitcast(f32)
                nc.scalar.activation(out=sq[:], in_=half,
                                     func=mybir.ActivationFunctionType.Square)
                if h == 0:
                    nc.vector.tensor_add(sa[:], sq[:, 0:512], sq[:, 512:1024])
                else:
                    nc.vector.tensor_add(sa[:], sa[:], sq[:, 0:512])
                    nc.vector.tensor_add(sa[:], sa[:], sq[:, 512:1024])
                nc.vector.tensor_add(sa[:], sa[:], sq[:, 1024:1536])
                nc.vector.tensor_add(sa[:], sa[:], sq[:, 1536:2048])
            nc.vector.tensor_add(sqacc[:], sa[:, 0:128], sa[:, 128:256])
            nc.vector.tensor_add(sqacc[:], sqacc[:], sa[:, 256:384])
            nc.vector.tensor_add(sqacc[:], sqacc[:], sa[:, 384:512])
            return sqacc

        def rownorm_post(t, sqacc):
            """Full 128x128 block transpose (d<->n) then free-axis reduce:
            a partition reduction built only from DVE ops."""
            red = rn_p.tile([128, 128], bf16, tag="rnred", name=f"rnred{t}")
            for i in range(4):
                for j in range(4):
                    nc.vector.transpose(
                        red[i * 32:(i + 1) * 32, j * 32:(j + 1) * 32],
                        sqacc[j * 32:(j + 1) * 32, i * 32:(i + 1) * 32])
            nsq = rn_p.tile([128, 1], f32, tag="rnt", name=f"rnt{t}")
            nc.vector.reduce_sum(nsq[:], red[:], axis=mybir.AxisListType.X)
            inv_n = rn_p.tile([128, 1], f32, tag="invn", name=f"invn{t}")
            nc.scalar.sqrt(inv_n[:], nsq[:])
            nc.vector.reciprocal(inv_n[:], inv_n[:])
            return inv_n

        def epilogue(t, pA, pB, inv_n):
            ep = rn_p.tile([128, 24], f32, tag="ep", name=f"ep{t}")
            mxA, mxB = ep[:, 0:8], ep[:, 8:16]
            d01, c01 = ep[:, 16:17], ep[:, 18:20]
            epi = rn_p.tile([128, 24], u32, tag="epi", name=f"epi{t}")
            ixA, ixB, msk = epi[:, 0:8], epi[:, 8:16], epi[:, 16:17]
            # DVE may read only one PSUM operand per op: cols 0:2 stage
            # through SBUF before the subtract
            nc.vector.tensor_mul(pA[:], pA[:], inv_cb[:, 0:512])
            nc.vector.tensor_mul(pB[:], pB[:], inv_cb[:, 512:C])
            nc.vector.tensor_copy(c01, pA[:, 0:2])
            nc.vector.tensor_sub(d01, c01[:, 1:2], c01[:, 0:1])
            nc.vector.tensor_mul(d01, d01, inv_n[:])
            nc.vector.tensor_scalar(
                out=eta_acc[:, t:t + 1], in0=d01, scalar1=0.25, scalar2=0.5,
                op0=mybir.AluOpType.mult, op1=mybir.AluOpType.add)
            nc.vector.max(out=mxA, in_=pA[:])
            nc.vector.max(out=mxB, in_=pB[:])
            nc.vector.max_index(out=ixA, in_max=mxA, in_values=pA[:])
            nc.vector.max_index(out=ixB, in_max=mxB, in_values=pB[:])
            nc.vector.tensor_scalar_add(ixB[:, 0:1], ixB[:, 0:1], 512)
            nc.vector.tensor_tensor(out=msk, in0=mxA[:, 0:1], in1=mxB[:, 0:1],
                                    op=mybir.AluOpType.is_ge)
            nc.vector.tensor_copy(preds_acc[:, t:t + 1], ixB[:, 0:1])
            nc.vector.copy_predicated(preds_acc[:, t:t + 1], msk, ixA[:, 0:1])

        def alloc_ps(t):
            if t == ROT:
                # the pn banks are dead once inv_cb is broadcast (just before
                # the rotation ends), so the first phase-2 tile accumulates
                # there instead of waiting for a rotation tile's epilogue
                pA = pn_p.tile([128, 512], f32, tag="pn0", name=f"pA{t}")
                pB = pn_p.tile([128, 512], f32, tag="pn1", name=f"pB{t}")
            else:
                pA = psA_p.tile([128, 512], f32, tag="pA", name=f"pA{t}")
                pB = psB_p.tile([128, 512], f32, tag="pB", name=f"pB{t}")
            return pA, pB

        def mm_pair(t, dc, first, last):
            pA, pB = rot_ps[t]
            lhs = hx_tiles[t][:, dc * 128:(dc + 1) * 128]
            nc.tensor.matmul(pA[:], lhs, am_tiles[dc][:, 0:512],
                             start=first, stop=last)
            nc.tensor.matmul(pB[:], lhs, am_tiles[dc][:, 512:C],
                             start=first, stop=last)

        # ================= phase 1: staggered rotation =================
        # DMA order (one serial resource): am0, hx0, am1, hx1, am2..5, hx2,
        # am6..31. Act/DVE norm work and PE matmul rounds are emitted in the
        # same chunk order so every engine's in-order stream is paced by the
        # arrival of its own inputs.
        load_am(0)
        load_hx(0)
        rot_ps = {}
        rot_sqacc = {}
        for t in range(ROT):
            rot_ps[t] = alloc_ps(t)
        for dc in range(DCH):
            if dc > 0:
                load_am(dc)
            for t, after in HX_AFTER.items():
                if after == dc:
                    load_hx(t)
            amnorm_chunk(dc)
            if dc in PRE_AT:
                t_pre = PRE_AT[dc]
                rot_sqacc[t_pre] = rownorm_pre(t_pre)
            for t in range(ROT):
                if dc == ENTRY[t]:
                    mm_pair(t, dc, first=True, last=False)
                    for wdc in range(ENTRY[t]):   # wrap: missed chunks are
                        mm_pair(t, wdc, first=False, last=False)  # resident
                elif dc > ENTRY[t]:
                    mm_pair(t, dc, first=False, last=(dc == DCH - 1))

        pair = sq_pairs.pop('last')
        nc.tensor.matmul(pn0[0:1, :], ones_b[:], pair[:, 0:512],
                         start=False, stop=True)
        nc.tensor.matmul(pn1[0:1, :], ones_b[:], pair[:, 512:C],
                         start=False, stop=True)
        # ---- finish am norms -> inv_cb (halves pipelined) ----
        for h, pn in ((0, pn0), (1, pn1)):
            cols = slice(h * 512, (h + 1) * 512)
            nc.scalar.sqrt(inv_c1[:, cols], pn[0:1, :])
            nc.vector.reciprocal(inv_c1[:, cols], inv_c1[:, cols])
            nc.tensor.matmul(pn[:], ones_c[:], inv_c1[:, cols],
                             start=True, stop=True)
            nc.vector.tensor_copy(inv_cb[:, cols], pn[:])

        # rotation tiles: rownorm back half + epilogue
        for t in range(ROT):
            inv_n = rownorm_post(t, rot_sqacc[t])
            pA, pB = rot_ps[t]
            epilogue(t, pA, pB, inv_n)

        # ================= phase 2: serial tiles =================
        for t in range(ROT, NT):
            load_hx(t)
            sqacc = rownorm_pre(t)
            inv_n = rownorm_post(t, sqacc)
            rot_ps[t] = alloc_ps(t)
            for dc in range(DCH):
                mm_pair(t, dc, first=(dc == 0), last=(dc == DCH - 1))
            epilogue(t, *rot_ps[t], inv_n)
            if t == 7:
                nc.sync.dma_start(preds_o[:, 0:8], preds_acc[:, 0:8])
                nc.sync.dma_start(eta_o[:, 0:8], eta_acc[:, 0:8])

        nc.sync.dma_start(preds_o[:, 8:NT], preds_acc[:, 8:NT])
        nc.sync.dma_start(eta_o[:, 8:NT], eta_acc[:, 8:NT])

    _split_multiwait(nc)
    return nc


_CACHE = {}


def kernel(hvs: np.ndarray, am: np.ndarray):
    hvs = np.asarray(hvs, dtype=np.float32)
    am = np.asarray(am, dtype=np.float32)
    assert hvs.shape == (N_FULL, D) and am.shape == (C, D)

    if "nc" not in _CACHE:
        _CACHE["nc"] = build_nc()
    nc = _CACHE["nc"]

    amT = np.ascontiguousarray(am.T)                      # [D, C]
    in_maps = []
    for r in range(N_CORES):
        shard = hvs[r * NS:(r + 1) * NS]                  # [NS, D]
        hvsT = np.ascontiguousarray(shard.T)              # [D, NS]
        in_maps.append({"hvsT": hvsT, "amT": amT})

    res = run_bass_kernel_spmd(nc, in_maps, core_ids=list(range(N_CORES)))

    preds = np.empty(N_FULL, dtype=np.int32)
    eta = np.empty(N_FULL, dtype=np.float32)
    for r in range(N_CORES):
        p = res.results[r]["preds"]                       # [128, NT] u32
        e = res.results[r]["eta"]                         # [128, NT] f32
        preds[r * NS:(r + 1) * NS] = p.T.ravel().astype(np.int32)
        eta[r * NS:(r + 1) * NS] = e.T.ravel()
    return preds, eta


# revision 4
# speedup vs baseline: 1.2418x; 1.0565x over previous
"""HDModel retrieval kernel for 8x TRN2 NeuronCores.

reference:
    sims  = l2norm(hvs) @ l2norm(am).T        # [N, C] cosine sims
    preds = argmax(sims, axis=1)              # int32 [N]
    eta   = (sims[:,1]-sims[:,0])*0.25 + 0.5  # f32 [N]

Data-parallel over N, am replicated, no cross-core comms.
309 us (original baseline) -> 263 us (f32r rework) -> ~249 us (fp16).

fp16 input staging: the host casts hvs/am to float16 (2^-11 mantissa, the
same noise class as the PE's f32r/tf32 compute: measured 11-13 argmax
flips vs f32r's 5 and the original baseline's 9, eta err ~2e-5 vs the
2e-2 gate). This halves every input byte, which halves the serialized
DMA stream that used to dominate the startup (the am load drops 50->25
us), making the kernel PE-bound nearly start to finish. hvs is also
pre-arranged on host so each n-tile is one contiguous block (fp16 rows
would otherwise fall under the <512B-run DMA penalty). The final tile
runs its two C-halves back to back instead of interleaved, hiding the
A-half epilogue under the B-half accumulation.

Design (engine budget: PE sims floor is 16 tiles x 64 f32r matmuls x
512 cols x 0.42 ns = ~219 us; everything else hides behind it):
  - PE runs the f32r sims matmuls plus only ~7 us of am-norm column
    reduction (bf16 ones-matmuls over pair-summed squares) and two fp32
    outer-product broadcasts. The old per-tile gram matmuls (~27 us) and
    full per-chunk norm matmuls (~14 us) are gone.
  - am col norms: Act squares each arriving am chunk (bf16); DVE adds
    chunk pairs; a [1,512]-row ones-matmul accumulates the pairs into two
    psum banks; after the last chunk: Act sqrt -> DVE reciprocal -> fp32
    ones outer-product rebroadcasts 1/|am_c| to all 128 partitions.
  - row norms: Act squares hx halves (bf16) -> DVE grouped adds down to
    sqacc [128 d, 128 n] -> full 128x128 transpose as 16 StreamTranspose
    32x32 blocks (a partition reduction built from DVE ops only; the
    gpsimd partition_all_reduce ucode does not compile on this backend)
    -> free-axis reduce -> sqrt -> reciprocal = 1/|x_n| on n-partitions.
  - Startup: the serialized DMA stream (am is 50 us of it) overlaps a
    staggered 3-tile chunk rotation (entries 0/2/6); a tile entering at
    round E first back-fills its missed chunks 0..E-1, which are already
    resident (psum accumulation order is irrelevant). The first phase-2
    tile accumulates in the pn banks, dead after the inv_c broadcast, so
    it starts without waiting for any rotation epilogue (this also keeps
    the PE p-state ramp warm into phase 2).
  - Epilogue scales psum in place (no [128,1024] sims sbuf tile), does
    per-half max/max_index and a compare/select merge (tie prefers half
    A = lower class index, matching jnp.argmax first-max semantics).
    eta's columns 0,1 stage through SBUF (single-PSUM-read rule).

This walrus build encodes ONE sync wait per TPB instruction; Tile attaches
several, so a post-pass splits multi-wait instructions into single-wait
same-engine NoOps (see _split_multiwait).
"""
import numpy as np
from contextlib import ExitStack

import concourse.bass as bass
import concourse.mybir as mybir
import concourse.tile as tile
from concourse.bass_utils import run_bass_kernel_spmd

f32 = mybir.dt.float32
f16 = mybir.dt.float16
bf16 = mybir.dt.bfloat16
u32 = mybir.dt.uint32

N_CORES = 8
N_FULL, D, C = 16384, 4096, 1024
NS = N_FULL // N_CORES          # 2048 rows per core
NT = NS // 128                  # 16 n-tiles
DCH = D // 128                  # 32 d-chunks
EPS = 1e-8
ROT = 3                         # tiles in the startup rotation
ENTRY = [0, 2, 6]               # rotation entry chunk per tile
HX_AFTER = {1: 1, 2: 5}         # dma hx[t] right after am chunk HX_AFTER[t]
PRE_AT = {2: 0, 7: 1, 11: 2}    # am chunk -> tile: emit rownorm_pre here


def _split_multiwait(nc):
    """Split multi-wait instructions into single-wait NoOps (walrus limit)."""
    ctr = [0]

    def mk_nop(engine, wait=None, update=None):
        ctr[0] += 1
        nop = mybir.InstNoOp(name=f"mwsplit_{ctr[0]}", ins=[], outs=[])
        nop.engine = engine
        nop.sync_info = mybir.SyncInfo(
            on_wait=[wait] if wait is not None else [],
            on_update=[update] if update is not None else [],
        )
        return nop

    for f in nc.m.functions:
        for bb in f.blocks:
            new = []
            changed = False
            for inst in bb.instructions:
                si = inst.sync_info
                if si is None:
                    new.append(inst)
                    continue
                waits = list(si.on_wait)
                updates = list(si.on_update)
                pre, post = [], []
                if len(waits) > 1:
                    pre = [mk_nop(inst.engine, wait=w) for w in waits[:-1]]
                    waits = waits[-1:]
                if len(updates) > 1 and type(inst).__name__ != "InstDMACopy":
                    post = [mk_nop(inst.engine, update=u) for u in updates[1:]]
                    updates = updates[:1]
                if pre or post:
                    inst.sync_info = mybir.SyncInfo(on_wait=waits, on_update=updates)
                    new.extend(pre)
                    new.append(inst)
                    new.extend(post)
                    changed = True
                else:
                    new.append(inst)
            if changed:
                bb.instructions = new


def build_nc():
    nc = bass.Bass()
    hvsX = nc.declare_dram_parameter("hvsX", [NT * 128, D], f16, isOutput=False)
    amT = nc.declare_dram_parameter("amT", [D, C], f16, isOutput=False)
    preds_o = nc.declare_dram_parameter("preds", [128, NT], u32, isOutput=True)
    eta_o = nc.declare_dram_parameter("eta", [128, NT], f32, isOutput=True)

    with tile.TileContext(nc) as tc, ExitStack() as ctx:
        am_p = ctx.enter_context(tc.tile_pool(name="am", bufs=1))
        hx_p = ctx.enter_context(tc.tile_pool(name="hx", bufs=3))
        sq_p = ctx.enter_context(tc.tile_pool(name="sq", bufs=1))
        sqam_p = ctx.enter_context(tc.tile_pool(name="sqam", bufs=2))
        nrm_p = ctx.enter_context(tc.tile_pool(name="nrm", bufs=1))
        rn_p = ctx.enter_context(tc.tile_pool(name="rn", bufs=2))
        rna_p = ctx.enter_context(tc.tile_pool(name="rna", bufs=4))
        acc_p = ctx.enter_context(tc.tile_pool(name="acc", bufs=1))
        psA_p = ctx.enter_context(tc.tile_pool(name="psA", bufs=3, space="PSUM"))
        psB_p = ctx.enter_context(tc.tile_pool(name="psB", bufs=3, space="PSUM"))
        pn_p = ctx.enter_context(tc.tile_pool(name="pn", bufs=1, space="PSUM"))

        inv_cb = nrm_p.tile([128, C], f32)        # 1/|am_c|, all partitions
        inv_c1 = nrm_p.tile([1, C], f32)          # staging row for broadcast
        ones_b = nrm_p.tile([128, 1], bf16)       # matmul reduction vector
        ones_c = nrm_p.tile([1, 128], f32)        # broadcast outer-product lhs
        preds_acc = acc_p.tile([128, NT], u32)
        eta_acc = acc_p.tile([128, NT], f32)
        nc.vector.memset(ones_b[:], 1.0)
        nc.vector.memset(ones_c[:], 1.0)

        am_tiles = []
        hx_tiles = {}

        def load_hx(t):
            """DMA one pre-arranged n-tile (contiguous fp16), quartered."""
            hx = hx_p.tile([128, D], f16, tag="hx", name=f"hx{t}")
            rows = hvsX[t * 128:(t + 1) * 128, :]
            for k in range(4):
                cols = slice(k * (D // 4), (k + 1) * (D // 4))
                nc.sync.dma_start(hx[:, cols], rows[:, cols])
            hx_tiles[t] = hx

        def load_am(dc):
            t = am_p.tile([128, C], f16, name=f"am{dc}")
            nc.sync.dma_start(t[:], amT[dc * 128:(dc + 1) * 128, :])
            am_tiles.append(t)

        # full-height tiles: row 0 holds the column-sum accumulation; after
        # the sqrt/recip read, the same banks take the broadcast outer product
        pn0 = pn_p.tile([128, 512], f32, tag="pn0", name="pn0")
        pn1 = pn_p.tile([128, 512], f32, tag="pn1", name="pn1")
        sq_pairs = {}

        def amnorm_chunk(dc):
            """Act square; on odd chunks pair-add (bf16) and accumulate the
            pair into pn via two 512-wide bf16 ones-matmuls (0.43 us PE per
            pair ~= the rotation's per-chunk slack vs the am arrival rate)."""
            sq = sqam_p.tile([128, C], bf16, tag="sqam", name=f"sqam{dc}")
            nc.scalar.activation(out=sq[:], in_=am_tiles[dc][:],
                                 func=mybir.ActivationFunctionType.Square)
            sq_pairs[dc] = sq
            if dc % 2 == 1:
                pair = sq_pairs.pop(dc - 1)
                nc.vector.tensor_add(pair[:], pair[:], sq[:])
                k = dc // 2
                if k == DCH // 2 - 1:
                    sq_pairs['last'] = pair   # emit after the rounds: keeps
                    return                    # round 31 off this chunk's
                nc.tensor.matmul(pn0[0:1, :], ones_b[:], pair[:, 0:512],     # Act/DVE chain
                                 start=(k == 0), stop=False)
                nc.tensor.matmul(pn1[0:1, :], ones_b[:], pair[:, 512:C],
                                 start=(k == 0), stop=False)

        def rownorm_pre(t):
            """Act square + DVE grouped adds -> sqacc [128 d, 128 n] bf16."""
            hx = hx_tiles[t]
            sq = sq_p.tile([128, D // 2], bf16, tag="sqhx", name=f"sqhx{t}")
            sa = rn_p.tile([128, 512], bf16, tag="rnsa", name=f"rnsa{t}")
            sqacc = rna_p.tile([128, 128], bf16, tag="rnacc", name=f"rnacc{t}")
            for h in range(2):
                half = hx[:, h * (D // 2):(h + 1) * (D // 2)]
                nc.scalar.activation(out=sq[:], in_=half,
                                     func=mybir.ActivationFunctionType.Square)
                if h == 0:
                    nc.vector.tensor_add(sa[:], sq[:, 0:512], sq[:, 512:1024])
                else:
                    nc.vector.tensor_add(sa[:], sa[:], sq[:, 0:512])
                    nc.vector.tensor_add(sa[:], sa[:], sq[:, 512:1024])
                nc.vector.tensor_add(sa[:], sa[:], sq[:, 1024:1536])
                nc.vector.tensor_add(sa[:], sa[:], sq[:, 1536:2048])
            nc.vector.tensor_add(sqacc[:], sa[:, 0:128], sa[:, 128:256])
            nc.vector.tensor_add(sqacc[:], sqacc[:], sa[:, 256:384])
            nc.vector.tensor_add(sqacc[:], sqacc[:], sa[:, 384:512])
            return sqacc

        def rownorm_post(t, sqacc):
            """Full 128x128 block transpose (d<->n) then free-axis reduce:
            a partition reduction built only from DVE ops."""
            red = rn_p.tile([128, 128], bf16, tag="rnred", name=f"rnred{t}")
            for i in range(4):
                for j in range(4):
                    nc.vector.transpose(
                        red[i * 32:(i + 1) * 32, j * 32:(j + 1) * 32],
                        sqacc[j * 32:(j + 1) * 32, i * 32:(i + 1) * 32])
            nsq = rn_p.tile([128, 1], f32, tag="rnt", name=f"rnt{t}")
            nc.vector.reduce_sum(nsq[:], red[:], axis=mybir.AxisListType.X)
            inv_n = rn_p.tile([128, 1], f32, tag="invn", name=f"invn{t}")
            nc.scalar.sqrt(inv_n[:], nsq[:])
            nc.vector.reciprocal(inv_n[:], inv_n[:])
            return inv_n

        def epilogue(t, pA, pB, inv_n):
            ep = rn_p.tile([128, 24], f32, tag="ep", name=f"ep{t}")
            mxA, mxB = ep[:, 0:8], ep[:, 8:16]
            d01, c01 = ep[:, 16:17], ep[:, 18:20]
            epi = rn_p.tile([128, 24], u32, tag="epi", name=f"epi{t}")
            ixA, ixB, msk = epi[:, 0:8], epi[:, 8:16], epi[:, 16:17]
            # DVE may read only one PSUM operand per op: cols 0:2 stage
            # through SBUF before the subtract
            nc.vector.tensor_mul(pA[:], pA[:], inv_cb[:, 0:512])
            nc.vector.tensor_mul(pB[:], pB[:], inv_cb[:, 512:C])
            nc.vector.tensor_copy(c01, pA[:, 0:2])
            nc.vector.tensor_sub(d01, c01[:, 1:2], c01[:, 0:1])
            nc.vector.tensor_mul(d01, d01, inv_n[:])
            nc.vector.tensor_scalar(
                out=eta_acc[:, t:t + 1], in0=d01, scalar1=0.25, scalar2=0.5,
                op0=mybir.AluOpType.mult, op1=mybir.AluOpType.add)
            nc.vector.max(out=mxA, in_=pA[:])
            nc.vector.max(out=mxB, in_=pB[:])
            nc.vector.max_index(out=ixA, in_max=mxA, in_values=pA[:])
            nc.vector.max_index(out=ixB, in_max=mxB, in_values=pB[:])
            nc.vector.tensor_scalar_add(ixB[:, 0:1], ixB[:, 0:1], 512)
            nc.vector.tensor_tensor(out=msk, in0=mxA[:, 0:1], in1=mxB[:, 0:1],
                                    op=mybir.AluOpType.is_ge)
            nc.vector.tensor_copy(preds_acc[:, t:t + 1], ixB[:, 0:1])
            nc.vector.copy_predicated(preds_acc[:, t:t + 1], msk, ixA[:, 0:1])

        def alloc_ps(t):
            if t == ROT:
                # the pn banks are dead once inv_cb is broadcast (just before
                # the rotation ends), so the first phase-2 tile accumulates
                # there instead of waiting for a rotation tile's epilogue
                pA = pn_p.tile([128, 512], f32, tag="pn0", name=f"pA{t}")
                pB = pn_p.tile([128, 512], f32, tag="pn1", name=f"pB{t}")
            else:
                pA = psA_p.tile([128, 512], f32, tag="pA", name=f"pA{t}")
                pB = psB_p.tile([128, 512], f32, tag="pB", name=f"pB{t}")
            return pA, pB

        def mm_pair(t, dc, first, last):
            pA, pB = rot_ps[t]
            lhs = hx_tiles[t][:, dc * 128:(dc + 1) * 128]
            nc.tensor.matmul(pA[:], lhs, am_tiles[dc][:, 0:512],
                             start=first, stop=last)
            nc.tensor.matmul(pB[:], lhs, am_tiles[dc][:, 512:C],
                             start=first, stop=last)

        # ================= phase 1: staggered rotation =================
        # DMA order (one serial resource): am0, hx0, am1, hx1, am2..5, hx2,
        # am6..31. Act/DVE norm work and PE matmul rounds are emitted in the
        # same chunk order so every engine's in-order stream is paced by the
        # arrival of its own inputs.
        load_am(0)
        load_hx(0)
        rot_ps = {}
        rot_sqacc = {}
        for t in range(ROT):
            rot_ps[t] = alloc_ps(t)
        for dc in range(DCH):
            if dc > 0:
                load_am(dc)
            for t, after in HX_AFTER.items():
                if after == dc:
                    load_hx(t)
            amnorm_chunk(dc)
            if dc in PRE_AT:
                t_pre = PRE_AT[dc]
                rot_sqacc[t_pre] = rownorm_pre(t_pre)
            for t in range(ROT):
                if dc == ENTRY[t]:
                    mm_pair(t, dc, first=True, last=False)
                    for wdc in range(ENTRY[t]):   # wrap: missed chunks are
                        mm_pair(t, wdc, first=False, last=False)  # resident
                elif dc > ENTRY[t]:
                    mm_pair(t, dc, first=False, last=(dc == DCH - 1))

        pair = sq_pairs.pop('last')
        nc.tensor.matmul(pn0[0:1, :], ones_b[:], pair[:, 0:512],
                         start=False, stop=True)
        nc.tensor.matmul(pn1[0:1, :], ones_b[:], pair[:, 512:C],
                         start=False, stop=True)
        # ---- finish am norms -> inv_cb (halves pipelined) ----
        for h, pn in ((0, pn0), (1, pn1)):
            cols = slice(h * 512, (h + 1) * 512)
            nc.scalar.sqrt(inv_c1[:, cols], pn[0:1, :])
            nc.vector.reciprocal(inv_c1[:, cols], inv_c1[:, cols])
            nc.tensor.matmul(pn[:], ones_c[:], inv_c1[:, cols],
                             start=True, stop=True)
            nc.vector.tensor_copy(inv_cb[:, cols], pn[:])

        # rotation tiles: rownorm back half + epilogue
        for t in range(ROT):
            inv_n = rownorm_post(t, rot_sqacc[t])
            pA, pB = rot_ps[t]
            epilogue(t, pA, pB, inv_n)

        # ================= phase 2: serial tiles =================
        for t in range(ROT, NT):
            load_hx(t)
            sqacc = rownorm_pre(t)
            inv_n = rownorm_post(t, sqacc)
            pA, pB = alloc_ps(t)
            rot_ps[t] = (pA, pB)
            if t == NT - 1:
                # final tile: all pA matmuls first, then pB, so the A-half
                # epilogue runs under the B-half accumulation and only the
                # B-half chain remains after the last matmul
                hx = hx_tiles[t]
                for dc in range(DCH):
                    nc.tensor.matmul(pA[:], hx[:, dc * 128:(dc + 1) * 128],
                                     am_tiles[dc][:, 0:512],
                                     start=(dc == 0), stop=(dc == DCH - 1))
                ep = rn_p.tile([128, 24], f32, tag="ep", name=f"ep{t}")
                mxA, mxB = ep[:, 0:8], ep[:, 8:16]
                d01, c01 = ep[:, 16:17], ep[:, 18:20]
                epi = rn_p.tile([128, 24], u32, tag="epi", name=f"epi{t}")
                ixA, ixB, msk = epi[:, 0:8], epi[:, 8:16], epi[:, 16:17]
                nc.vector.tensor_mul(pA[:], pA[:], inv_cb[:, 0:512])
                nc.vector.tensor_copy(c01, pA[:, 0:2])
                nc.vector.tensor_sub(d01, c01[:, 1:2], c01[:, 0:1])
                nc.vector.tensor_mul(d01, d01, inv_n[:])
                nc.vector.tensor_scalar(
                    out=eta_acc[:, t:t + 1], in0=d01, scalar1=0.25,
                    scalar2=0.5, op0=mybir.AluOpType.mult,
                    op1=mybir.AluOpType.add)
                nc.vector.max(out=mxA, in_=pA[:])
                nc.vector.max_index(out=ixA, in_max=mxA, in_values=pA[:])
                for dc in range(DCH):
                    nc.tensor.matmul(pB[:], hx[:, dc * 128:(dc + 1) * 128],
                                     am_tiles[dc][:, 512:C],
                                     start=(dc == 0), stop=(dc == DCH - 1))
                nc.vector.tensor_mul(pB[:], pB[:], inv_cb[:, 512:C])
                nc.vector.max(out=mxB, in_=pB[:])
                nc.vector.max_index(out=ixB, in_max=mxB, in_values=pB[:])
                nc.vector.tensor_scalar_add(ixB[:, 0:1], ixB[:, 0:1], 512)
                nc.vector.tensor_tensor(out=msk, in0=mxA[:, 0:1],
                                        in1=mxB[:, 0:1],
                                        op=mybir.AluOpType.is_ge)
                nc.vector.tensor_copy(preds_acc[:, t:t + 1], ixB[:, 0:1])
                nc.vector.copy_predicated(preds_acc[:, t:t + 1], msk,
                                          ixA[:, 0:1])
            else:
                for dc in range(DCH):
                    mm_pair(t, dc, first=(dc == 0), last=(dc == DCH - 1))
                epilogue(t, *rot_ps[t], inv_n)
            if t == 7:
                nc.sync.dma_start(preds_o[:, 0:8], preds_acc[:, 0:8])
                nc.sync.dma_start(eta_o[:, 0:8], eta_acc[:, 0:8])

        nc.sync.dma_start(preds_o[:, 8:NT], preds_acc[:, 8:NT])
        nc.sync.dma_start(eta_o[:, 8:NT], eta_acc[:, 8:NT])

    _split_multiwait(nc)
    return nc


_CACHE = {}


def kernel(hvs: np.ndarray, am: np.ndarray):
    hvs = np.asarray(hvs, dtype=np.float32)
    am = np.asarray(am, dtype=np.float32)
    assert hvs.shape == (N_FULL, D) and am.shape == (C, D)

    if "nc" not in _CACHE:
        _CACHE["nc"] = build_nc()
    nc = _CACHE["nc"]

    amT = np.ascontiguousarray(am.T.astype(np.float16))   # [D, C] fp16
    in_maps = []
    for r in range(N_CORES):
        shard = hvs[r * NS:(r + 1) * NS].astype(np.float16)
        # hvsX[t*128+p, dc*128+j] = shard[t*128+j, dc*128+p]: each 128-row
        # block is exactly one SBUF tile's [d-part, (dc, n)] content, so the
        # device DMA is a contiguous copy (fp16 would otherwise fall under
        # the 512B-run DMA penalty)
        a = shard.reshape(NT, 128, DCH, 128)              # [t, j, dc, p]
        hvsX = np.ascontiguousarray(
            a.transpose(0, 3, 2, 1).reshape(NT * 128, D))
        in_maps.append({"hvsX": hvsX, "amT": amT})

    res = run_bass_kernel_spmd(nc, in_maps, core_ids=list(range(N_CORES)))

    preds = np.empty(N_FULL, dtype=np.int32)
    eta = np.empty(N_FULL, dtype=np.float32)
    for r in range(N_CORES):
        p = res.results[r]["preds"]                       # [128, NT] u32
        e = res.results[r]["eta"]                         # [128, NT] f32
        preds[r * NS:(r + 1) * NS] = p.T.ravel().astype(np.int32)
        eta[r * NS:(r + 1) * NS] = e.T.ravel()
    return preds, eta


# revision 5
# speedup vs baseline: 1.2487x; 1.0056x over previous
"""HDModel retrieval kernel for 8x TRN2 NeuronCores.

reference:
    sims  = l2norm(hvs) @ l2norm(am).T        # [N, C] cosine sims
    preds = argmax(sims, axis=1)              # int32 [N]
    eta   = (sims[:,1]-sims[:,0])*0.25 + 0.5  # f32 [N]

Data-parallel over N, am replicated, no cross-core comms.
309 us (original baseline) -> 263 us (f32r rework) -> ~249 us (fp16).

fp16 input staging: the host casts hvs/am to float16 (2^-11 mantissa, the
same noise class as the PE's f32r/tf32 compute: measured 11-13 argmax
flips vs f32r's 5 and the original baseline's 9, eta err ~2e-5 vs the
2e-2 gate). This halves every input byte, which halves the serialized
DMA stream that used to dominate the startup (the am load drops 50->25
us), making the kernel PE-bound nearly start to finish. hvs is also
pre-arranged on host so each n-tile is one contiguous block (fp16 rows
would otherwise fall under the <512B-run DMA penalty). The final tile
runs its two C-halves back to back instead of interleaved, hiding the
A-half epilogue under the B-half accumulation.

Design (engine budget: PE sims floor is 16 tiles x 64 f32r matmuls x
512 cols x 0.42 ns = ~219 us; everything else hides behind it):
  - PE runs the f32r sims matmuls plus only ~7 us of am-norm column
    reduction (bf16 ones-matmuls over pair-summed squares) and two fp32
    outer-product broadcasts. The old per-tile gram matmuls (~27 us) and
    full per-chunk norm matmuls (~14 us) are gone.
  - am col norms: Act squares each arriving am chunk (bf16); DVE adds
    chunk pairs; a [1,512]-row ones-matmul accumulates the pairs into two
    psum banks; after the last chunk: Act sqrt -> DVE reciprocal -> fp32
    ones outer-product rebroadcasts 1/|am_c| to all 128 partitions.
  - row norms: Act squares hx halves (bf16) -> DVE grouped adds down to
    sqacc [128 d, 128 n] -> full 128x128 transpose as 16 StreamTranspose
    32x32 blocks (a partition reduction built from DVE ops only; the
    gpsimd partition_all_reduce ucode does not compile on this backend)
    -> free-axis reduce -> sqrt -> reciprocal = 1/|x_n| on n-partitions.
  - Startup: the serialized DMA stream (am is 50 us of it) overlaps a
    staggered 3-tile chunk rotation (entries 0/2/6); a tile entering at
    round E first back-fills its missed chunks 0..E-1, which are already
    resident (psum accumulation order is irrelevant). The first phase-2
    tile accumulates in the pn banks, dead after the inv_c broadcast, so
    it starts without waiting for any rotation epilogue (this also keeps
    the PE p-state ramp warm into phase 2).
  - Epilogue scales psum in place (no [128,1024] sims sbuf tile), does
    per-half max/max_index and a compare/select merge (tie prefers half
    A = lower class index, matching jnp.argmax first-max semantics).
    eta's columns 0,1 stage through SBUF (single-PSUM-read rule).

This walrus build encodes ONE sync wait per TPB instruction; Tile attaches
several, so a post-pass splits multi-wait instructions into single-wait
same-engine NoOps (see _split_multiwait).
"""
import numpy as np
from contextlib import ExitStack

import concourse.bass as bass
import concourse.mybir as mybir
import concourse.tile as tile
from concourse.bass_utils import run_bass_kernel_spmd

f32 = mybir.dt.float32
f16 = mybir.dt.float16
bf16 = mybir.dt.bfloat16
u32 = mybir.dt.uint32

N_CORES = 8
N_FULL, D, C = 16384, 4096, 1024
NS = N_FULL // N_CORES          # 2048 rows per core
NT = NS // 128                  # 16 n-tiles
DCH = D // 128                  # 32 d-chunks
EPS = 1e-8
ROT = 3                         # tiles in the startup rotation
ENTRY = [0, 2, 6]               # rotation entry chunk per tile
HX_AFTER = {1: 1, 2: 5}         # dma hx[t] right after am chunk HX_AFTER[t]
PRE_AT = {2: 0, 7: 1, 11: 2}    # am chunk -> tile: emit rownorm_pre here


def _split_multiwait(nc):
    """Split multi-wait instructions into single-wait NoOps (walrus limit)."""
    ctr = [0]

    def mk_nop(engine, wait=None, update=None):
        ctr[0] += 1
        nop = mybir.InstNoOp(name=f"mwsplit_{ctr[0]}", ins=[], outs=[])
        nop.engine = engine
        nop.sync_info = mybir.SyncInfo(
            on_wait=[wait] if wait is not None else [],
            on_update=[update] if update is not None else [],
        )
        return nop

    for f in nc.m.functions:
        for bb in f.blocks:
            new = []
            changed = False
            for inst in bb.instructions:
                si = inst.sync_info
                if si is None:
                    new.append(inst)
                    continue
                waits = list(si.on_wait)
                updates = list(si.on_update)
                pre, post = [], []
                if len(waits) > 1:
                    pre = [mk_nop(inst.engine, wait=w) for w in waits[:-1]]
                    waits = waits[-1:]
                if len(updates) > 1 and type(inst).__name__ != "InstDMACopy":
                    post = [mk_nop(inst.engine, update=u) for u in updates[1:]]
                    updates = updates[:1]
                if pre or post:
                    inst.sync_info = mybir.SyncInfo(on_wait=waits, on_update=updates)
                    new.extend(pre)
                    new.append(inst)
                    new.extend(post)
                    changed = True
                else:
                    new.append(inst)
            if changed:
                bb.instructions = new


def build_nc():
    nc = bass.Bass()
    hvsX = nc.declare_dram_parameter("hvsX", [NT * 128, D], f16, isOutput=False)
    amT = nc.declare_dram_parameter("amT", [D, C], f16, isOutput=False)
    preds_o = nc.declare_dram_parameter("preds", [128, NT], u32, isOutput=True)
    eta_o = nc.declare_dram_parameter("eta", [128, NT], f32, isOutput=True)

    with tile.TileContext(nc) as tc, ExitStack() as ctx:
        am_p = ctx.enter_context(tc.tile_pool(name="am", bufs=1))
        hx_p = ctx.enter_context(tc.tile_pool(name="hx", bufs=3))
        sq_p = ctx.enter_context(tc.tile_pool(name="sq", bufs=1))
        sqam_p = ctx.enter_context(tc.tile_pool(name="sqam", bufs=2))
        nrm_p = ctx.enter_context(tc.tile_pool(name="nrm", bufs=1))
        rn_p = ctx.enter_context(tc.tile_pool(name="rn", bufs=2))
        rna_p = ctx.enter_context(tc.tile_pool(name="rna", bufs=4))
        acc_p = ctx.enter_context(tc.tile_pool(name="acc", bufs=1))
        psA_p = ctx.enter_context(tc.tile_pool(name="psA", bufs=3, space="PSUM"))
        psB_p = ctx.enter_context(tc.tile_pool(name="psB", bufs=3, space="PSUM"))
        pn_p = ctx.enter_context(tc.tile_pool(name="pn", bufs=1, space="PSUM"))

        inv_cb = nrm_p.tile([128, C], f32)        # 1/|am_c|, all partitions
        inv_c1 = nrm_p.tile([1, C], f32)          # staging row for broadcast
        ones_b = nrm_p.tile([128, 1], bf16)       # matmul reduction vector
        ones_c = nrm_p.tile([1, 128], f32)        # broadcast outer-product lhs
        preds_acc = acc_p.tile([128, NT], u32)
        eta_acc = acc_p.tile([128, NT], f32)
        nc.vector.memset(ones_b[:], 1.0)
        nc.vector.memset(ones_c[:], 1.0)

        am_tiles = []
        hx_tiles = {}

        def load_hx(t, interleave=None):
            """DMA one pre-arranged n-tile (contiguous fp16), quartered.
            interleave: optional callback run between quarters (lets the
            first am chunks slot between hx0's quarters so tile 0's early
            rounds aren't starved)."""
            hx = hx_p.tile([128, D], f16, tag="hx", name=f"hx{t}")
            rows = hvsX[t * 128:(t + 1) * 128, :]
            for k in range(4):
                cols = slice(k * (D // 4), (k + 1) * (D // 4))
                nc.sync.dma_start(hx[:, cols], rows[:, cols])
                if interleave:
                    interleave(k)
            hx_tiles[t] = hx

        def load_am(dc):
            t = am_p.tile([128, C], f16, name=f"am{dc}")
            nc.sync.dma_start(t[:], amT[dc * 128:(dc + 1) * 128, :])
            am_tiles.append(t)

        # full-height tiles: row 0 holds the column-sum accumulation; after
        # the sqrt/recip read, the same banks take the broadcast outer product
        pn0 = pn_p.tile([128, 512], f32, tag="pn0", name="pn0")
        pn1 = pn_p.tile([128, 512], f32, tag="pn1", name="pn1")
        sq_pairs = {}

        def amnorm_chunk(dc):
            """Act square; on odd chunks pair-add (bf16) and accumulate the
            pair into pn via two 512-wide bf16 ones-matmuls (0.43 us PE per
            pair ~= the rotation's per-chunk slack vs the am arrival rate)."""
            sq = sqam_p.tile([128, C], bf16, tag="sqam", name=f"sqam{dc}")
            nc.scalar.activation(out=sq[:], in_=am_tiles[dc][:],
                                 func=mybir.ActivationFunctionType.Square)
            sq_pairs[dc] = sq
            if dc % 2 == 1:
                pair = sq_pairs.pop(dc - 1)
                nc.vector.tensor_add(pair[:], pair[:], sq[:])
                k = dc // 2
                if k == DCH // 2 - 1:
                    sq_pairs['last'] = pair   # emit after the rounds: keeps
                    return                    # round 31 off this chunk's
                nc.tensor.matmul(pn0[0:1, :], ones_b[:], pair[:, 0:512],     # Act/DVE chain
                                 start=(k == 0), stop=False)
                nc.tensor.matmul(pn1[0:1, :], ones_b[:], pair[:, 512:C],
                                 start=(k == 0), stop=False)

        def rownorm_pre(t):
            """Act square + DVE grouped adds -> sqacc [128 d, 128 n] bf16."""
            hx = hx_tiles[t]
            sq = sq_p.tile([128, D // 2], bf16, tag="sqhx", name=f"sqhx{t}")
            sa = rn_p.tile([128, 512], bf16, tag="rnsa", name=f"rnsa{t}")
            sqacc = rna_p.tile([128, 128], bf16, tag="rnacc", name=f"rnacc{t}")
            for h in range(2):
                half = hx[:, h * (D // 2):(h + 1) * (D // 2)]
                nc.scalar.activation(out=sq[:], in_=half,
                                     func=mybir.ActivationFunctionType.Square)
                if h == 0:
                    nc.vector.tensor_add(sa[:], sq[:, 0:512], sq[:, 512:1024])
                else:
                    nc.vector.tensor_add(sa[:], sa[:], sq[:, 0:512])
                    nc.vector.tensor_add(sa[:], sa[:], sq[:, 512:1024])
                nc.vector.tensor_add(sa[:], sa[:], sq[:, 1024:1536])
                nc.vector.tensor_add(sa[:], sa[:], sq[:, 1536:2048])
            nc.vector.tensor_add(sqacc[:], sa[:, 0:128], sa[:, 128:256])
            nc.vector.tensor_add(sqacc[:], sqacc[:], sa[:, 256:384])
            nc.vector.tensor_add(sqacc[:], sqacc[:], sa[:, 384:512])
            return sqacc

        def rownorm_post(t, sqacc):
            """Full 128x128 block transpose (d<->n) then free-axis reduce:
            a partition reduction built only from DVE ops."""
            red = rn_p.tile([128, 128], bf16, tag="rnred", name=f"rnred{t}")
            for i in range(4):
                for j in range(4):
                    nc.vector.transpose(
                        red[i * 32:(i + 1) * 32, j * 32:(j + 1) * 32],
                        sqacc[j * 32:(j + 1) * 32, i * 32:(i + 1) * 32])
            nsq = rn_p.tile([128, 1], f32, tag="rnt", name=f"rnt{t}")
            nc.vector.reduce_sum(nsq[:], red[:], axis=mybir.AxisListType.X)
            inv_n = rn_p.tile([128, 1], f32, tag="invn", name=f"invn{t}")
            nc.scalar.sqrt(inv_n[:], nsq[:])
            nc.vector.reciprocal(inv_n[:], inv_n[:])
            return inv_n

        def epilogue(t, pA, pB, inv_n):
            ep = rn_p.tile([128, 24], f32, tag="ep", name=f"ep{t}")
            mxA, mxB = ep[:, 0:8], ep[:, 8:16]
            d01, c01 = ep[:, 16:17], ep[:, 18:20]
            epi = rn_p.tile([128, 24], u32, tag="epi", name=f"epi{t}")
            ixA, ixB, msk = epi[:, 0:8], epi[:, 8:16], epi[:, 16:17]
            # DVE may read only one PSUM operand per op: cols 0:2 stage
            # through SBUF before the subtract
            nc.vector.tensor_mul(pA[:], pA[:], inv_cb[:, 0:512])
            nc.vector.tensor_mul(pB[:], pB[:], inv_cb[:, 512:C])
            nc.vector.tensor_copy(c01, pA[:, 0:2])
            nc.vector.tensor_sub(d01, c01[:, 1:2], c01[:, 0:1])
            nc.vector.tensor_mul(d01, d01, inv_n[:])
            nc.vector.tensor_scalar(
                out=eta_acc[:, t:t + 1], in0=d01, scalar1=0.25, scalar2=0.5,
                op0=mybir.AluOpType.mult, op1=mybir.AluOpType.add)
            nc.vector.max(out=mxA, in_=pA[:])
            nc.vector.max(out=mxB, in_=pB[:])
            nc.vector.max_index(out=ixA, in_max=mxA, in_values=pA[:])
            nc.vector.max_index(out=ixB, in_max=mxB, in_values=pB[:])
            nc.vector.tensor_scalar_add(ixB[:, 0:1], ixB[:, 0:1], 512)
            nc.vector.tensor_tensor(out=msk, in0=mxA[:, 0:1], in1=mxB[:, 0:1],
                                    op=mybir.AluOpType.is_ge)
            nc.vector.tensor_copy(preds_acc[:, t:t + 1], ixB[:, 0:1])
            nc.vector.copy_predicated(preds_acc[:, t:t + 1], msk, ixA[:, 0:1])

        def alloc_ps(t):
            if t == ROT:
                # the pn banks are dead once inv_cb is broadcast (just before
                # the rotation ends), so the first phase-2 tile accumulates
                # there instead of waiting for a rotation tile's epilogue
                pA = pn_p.tile([128, 512], f32, tag="pn0", name=f"pA{t}")
                pB = pn_p.tile([128, 512], f32, tag="pn1", name=f"pB{t}")
            else:
                pA = psA_p.tile([128, 512], f32, tag="pA", name=f"pA{t}")
                pB = psB_p.tile([128, 512], f32, tag="pB", name=f"pB{t}")
            return pA, pB

        def mm_pair(t, dc, first, last):
            pA, pB = rot_ps[t]
            lhs = hx_tiles[t][:, dc * 128:(dc + 1) * 128]
            nc.tensor.matmul(pA[:], lhs, am_tiles[dc][:, 0:512],
                             start=first, stop=last)
            nc.tensor.matmul(pB[:], lhs, am_tiles[dc][:, 512:C],
                             start=first, stop=last)

        # ================= phase 1: staggered rotation =================
        # DMA order (one serial resource): am0, hx0, am1, hx1, am2..5, hx2,
        # am6..31. Act/DVE norm work and PE matmul rounds are emitted in the
        # same chunk order so every engine's in-order stream is paced by the
        # arrival of its own inputs.
        load_am(0)
        load_hx(0, interleave=lambda k: load_am(k + 1) if k < 3 else None)
        rot_ps = {}
        rot_sqacc = {}
        for t in range(ROT):
            rot_ps[t] = alloc_ps(t)
        for dc in range(DCH):
            if dc > 3:
                load_am(dc)
            for t, after in HX_AFTER.items():
                if after == dc:
                    load_hx(t)
            amnorm_chunk(dc)
            if dc in PRE_AT:
                t_pre = PRE_AT[dc]
                rot_sqacc[t_pre] = rownorm_pre(t_pre)
            for t in range(ROT):
                if dc == ENTRY[t]:
                    mm_pair(t, dc, first=True, last=False)
                    for wdc in range(ENTRY[t]):   # wrap: missed chunks are
                        mm_pair(t, wdc, first=False, last=False)  # resident
                elif dc > ENTRY[t]:
                    mm_pair(t, dc, first=False, last=(dc == DCH - 1))

        pair = sq_pairs.pop('last')
        nc.tensor.matmul(pn0[0:1, :], ones_b[:], pair[:, 0:512],
                         start=False, stop=True)
        nc.tensor.matmul(pn1[0:1, :], ones_b[:], pair[:, 512:C],
                         start=False, stop=True)
        # ---- finish am norms -> inv_cb (halves pipelined) ----
        for h, pn in ((0, pn0), (1, pn1)):
            cols = slice(h * 512, (h + 1) * 512)
            nc.scalar.sqrt(inv_c1[:, cols], pn[0:1, :])
            nc.vector.reciprocal(inv_c1[:, cols], inv_c1[:, cols])
            nc.tensor.matmul(pn[:], ones_c[:], inv_c1[:, cols],
                             start=True, stop=True)
            nc.vector.tensor_copy(inv_cb[:, cols], pn[:])

        # rotation tiles: rownorm back half + epilogue
        for t in range(ROT):
            inv_n = rownorm_post(t, rot_sqacc[t])
            pA, pB = rot_ps[t]
            epilogue(t, pA, pB, inv_n)

        # ================= phase 2: serial tiles =================
        for t in range(ROT, NT):
            load_hx(t)
            sqacc = rownorm_pre(t)
            inv_n = rownorm_post(t, sqacc)
            pA, pB = alloc_ps(t)
            rot_ps[t] = (pA, pB)
            if t == NT - 1:
                # final tile: all pA matmuls first, then pB, so the A-half
                # epilogue runs under the B-half accumulation and only the
                # B-half chain remains after the last matmul
                hx = hx_tiles[t]
                for dc in range(DCH):
                    nc.tensor.matmul(pA[:], hx[:, dc * 128:(dc + 1) * 128],
                                     am_tiles[dc][:, 0:512],
                                     start=(dc == 0), stop=(dc == DCH - 1))
                ep = rn_p.tile([128, 24], f32, tag="ep", name=f"ep{t}")
                mxA, mxB = ep[:, 0:8], ep[:, 8:16]
                d01, c01 = ep[:, 16:17], ep[:, 18:20]
                epi = rn_p.tile([128, 24], u32, tag="epi", name=f"epi{t}")
                ixA, ixB, msk = epi[:, 0:8], epi[:, 8:16], epi[:, 16:17]
                nc.vector.tensor_mul(pA[:], pA[:], inv_cb[:, 0:512])
                nc.vector.tensor_copy(c01, pA[:, 0:2])
                nc.vector.tensor_sub(d01, c01[:, 1:2], c01[:, 0:1])
                nc.vector.tensor_mul(d01, d01, inv_n[:])
                nc.vector.tensor_scalar(
                    out=eta_acc[:, t:t + 1], in0=d01, scalar1=0.25,
                    scalar2=0.5, op0=mybir.AluOpType.mult,
                    op1=mybir.AluOpType.add)
                nc.vector.max(out=mxA, in_=pA[:])
                nc.vector.max_index(out=ixA, in_max=mxA, in_values=pA[:])
                for dc in range(DCH):
                    nc.tensor.matmul(pB[:], hx[:, dc * 128:(dc + 1) * 128],
                                     am_tiles[dc][:, 512:C],
                                     start=(dc == 0), stop=(dc == DCH - 1))
                nc.vector.tensor_mul(pB[:], pB[:], inv_cb[:, 512:C])
                nc.vector.max(out=mxB, in_=pB[:])
                nc.vector.max_index(out=ixB, in_max=mxB, in_values=pB[:])
                nc.vector.tensor_scalar_add(ixB[:, 0:1], ixB[:, 0:1], 512)
                nc.vector.tensor_tensor(out=msk, in0=mxA[:, 0:1],
                                        in1=mxB[:, 0:1],
                                        op=mybir.AluOpType.is_ge)
                nc.vector.tensor_copy(preds_acc[:, t:t + 1], ixB[:, 0:1])
                nc.vector.copy_predicated(preds_acc[:, t:t + 1], msk,
                                          ixA[:, 0:1])
            else:
                for dc in range(DCH):
                    mm_pair(t, dc, first=(dc == 0), last=(dc == DCH - 1))
                epilogue(t, *rot_ps[t], inv_n)
            if t == 7:
                nc.sync.dma_start(preds_o[:, 0:8], preds_acc[:, 0:8])
                nc.sync.dma_start(eta_o[:, 0:8], eta_acc[:, 0:8])

        nc.sync.dma_start(preds_o[:, 8:NT], preds_acc[:, 8:NT])
        nc.sync.dma_start(eta_o[:, 8:NT], eta_acc[:, 8:NT])

    _split_multiwait(nc)
    return nc


_CACHE = {}


def kernel(hvs: np.ndarray, am: np.ndarray):
    hvs = np.asarray(hvs, dtype=np.float32)
    am = np.asarray(am, dtype=np.float32)
    assert hvs.shape == (N_FULL, D) and am.shape == (C, D)

    if "nc" not in _CACHE:
        _CACHE["nc"] = build_nc()
    nc = _CACHE["nc"]

    amT = np.ascontiguousarray(am.T.astype(np.float16))   # [D, C] fp16
    in_maps = []
    for r in range(N_CORES):
        shard = hvs[r * NS:(r + 1) * NS].astype(np.float16)
        # hvsX[t*128+p, dc*128+j] = shard[t*128+j, dc*128+p]: each 128-row
        # block is exactly one SBUF tile's [d-part, (dc, n)] content, so the
        # device DMA is a contiguous copy (fp16 would otherwise fall under
        # the 512B-run DMA penalty)
        a = shard.reshape(NT, 128, DCH, 128)              # [t, j, dc, p]
        hvsX = np.ascontiguousarray(
            a.transpose(0, 3, 2, 1).reshape(NT * 128, D))
        in_maps.append({"hvsX": hvsX, "amT": amT})

    res = run_bass_kernel_spmd(nc, in_maps, core_ids=list(range(N_CORES)))

    preds = np.empty(N_FULL, dtype=np.int32)
    eta = np.empty(N_FULL, dtype=np.float32)
    for r in range(N_CORES):
        p = res.results[r]["preds"]                       # [128, NT] u32
        e = res.results[r]["eta"]                         # [128, NT] f32
        preds[r * NS:(r + 1) * NS] = p.T.ravel().astype(np.int32)
        eta[r * NS:(r + 1) * NS] = e.T.ravel()
    return preds, eta
